# revision 7
# baseline (speedup 1.0000x reference)
"""Trainium2 Bass kernel for nn_AnomalyDetector (multi-modal encoder + 2-layer
LSTM + normalizing flows + decoders + anomaly scores).

Data-parallel over 8 NeuronCores: batch 16384 -> 2048 per core. All on-chip
activations use a transposed layout [feature_on_partition, sample_on_free] so
the per-step LSTM matmuls need no transposes. Compute in bf16 with f32 PSUM
accumulation; outputs staged to one [830, 2048] f32 DRAM tensor per core.
"""

import numpy as np
import ml_dtypes

import concourse.bass as bass
import concourse.mybir as mybir
from concourse import bacc
from concourse.tile import TileContext
from concourse.bass import ts
from concourse.bass_utils import run_bass_kernel_spmd

BF16 = mybir.dt.bfloat16
F32 = mybir.dt.float32
AF = mybir.ActivationFunctionType

B = 16384
NCORES = 8
S = B // NCORES          # samples per core (2048)
T = 64
HID = 128
LAT = 64
DIMS = {"physical": 32, "orbital": 6, "signature": 256, "temporal": 16}
MODS = ["physical", "orbital", "signature", "temporal"]
NCH = 4                  # moving-sample chunks of 512
CH = S // NCH            # 512

# ------------------------------------------------------------------
# Output row map for the per-core OUT tensor [830, S] (feature, sample)
# ------------------------------------------------------------------
ROW_ENC = 0        # 4 x 64  (phys, orb, sig, temp)
ROW_TRF = 256      # 4 x 64
ROW_REC = 512      # 32, 6, 256, 16
REC_OFF = {"physical": 512, "orbital": 544, "signature": 550, "temporal": 806}
ROW_LD = 822       # 4 rows (phys, orb, sig, temp)
ROW_SC = 826       # 4 rows
NROWS = 830

# ------------------------------------------------------------------
# Weight blob (bf16) / param blob (f32) layouts: name -> (r0, nr, c0, nc)
# ------------------------------------------------------------------
def _build_wspec():
    spec = {}
    col = 0

    def add(name, r0, nr, nc):
        nonlocal col
        spec[name] = (r0, nr, col, nc)
        col += nc

    add("wih1", 0, 128, 512)      # 4x replicated along 32-row groups
    add("whh1", 0, 128, 512)
    add("wih2", 0, 128, 512)
    add("whh2", 0, 128, 512)
    add("wtp", 0, 128, 64)
    add("e1p", 0, 32, 128)
    add("e2p", 0, 128, 64)
    add("e1o", 0, 6, 128)
    add("e2o", 0, 128, 64)
    add("e1s_a", 0, 128, 128)
    add("e1s_b", 0, 128, 128)
    add("e2s", 0, 128, 64)
    add("fw0", 0, 128, 128)
    add("fw1", 0, 128, 128)
    add("fw2", 0, 128, 128)
    add("ones2", 0, 128, 2)
    add("d1p", 0, 64, 128)
    add("d1o", 64, 64, 128)
    add("d1s", 0, 64, 128)
    add("d1t", 64, 64, 128)
    add("d2p", 0, 128, 32)
    add("d2o", 0, 128, 6)
    add("d2s", 0, 128, 256)
    add("d2t", 0, 128, 16)
    add("s1p", 0, 64, 128)
    add("s1o", 64, 64, 128)
    add("s1s", 0, 64, 128)
    add("s1t", 64, 64, 128)
    add("s2p", 0, 128, 1)
    add("s2o", 0, 128, 1)
    add("s2s", 0, 128, 1)
    add("s2t", 0, 128, 1)
    return spec, col


WSPEC, WCOL = _build_wspec()

# param blob columns (f32)
PB_L1G = 0      # cols 0-3: layer1 gate biases (i,f,g,o)
PB_L2G = 4      # cols 4-7
PB_E1 = {"physical": 8, "orbital": 9, "signature": 10}
PB_ZA = 11      # [b2_phys ; b2_orb]
PB_ZB = 12      # [b2_sig ; b_tp]
PB_FLOW = 13    # 13,14,15
PB_D1 = 16      # 16-19 dec l1 biases (p,o,s,t)
PB_S1 = 20      # 20-23 score l1 biases
PB_D2 = {"physical": 24, "orbital": 25, "signature": 26, "temporal": 28}  # sig uses 26,27
PB_S2 = 29      # 29-32, row 0 only
PB_LDW = 33     # rows 0-1: sum of flow log|det W|
PCOL = 34


def _np(x):
    return np.asarray(x, dtype=np.float32)


def build_blobs(params):
    """Pack all model parameters into WBLOB (bf16) and PBLOB (f32)."""
    wb = np.zeros((128, WCOL), np.float32)
    pb = np.zeros((128, PCOL), np.float32)

    def put(name, arr):
        r0, nr, c0, ncol = WSPEC[name]
        a = _np(arr)
        assert a.shape == (nr, ncol), (name, a.shape, (nr, ncol))
        wb[r0:r0 + nr, c0:c0 + ncol] = a

    lstm = params["lstm"]
    # layer 1 Wih replicated at row groups 0..3 (16 rows each, 32-aligned)
    wih1 = np.zeros((128, 512), np.float32)
    w = _np(lstm[0]["Wih"])  # [16, 512]
    for r in range(4):
        wih1[32 * r:32 * r + 16, :] = w
    put("wih1", wih1)
    put("whh1", _np(lstm[0]["Whh"]))
    put("wih2", _np(lstm[1]["Wih"]))
    put("whh2", _np(lstm[1]["Whh"]))
    put("wtp", _np(params["temporal_proj"]["W"]))
    put("e1p", _np(params["enc_physical"]["l1"]["W"]))
    put("e2p", _np(params["enc_physical"]["l2"]["W"]))
    put("e1o", _np(params["enc_orbital"]["l1"]["W"]))
    put("e2o", _np(params["enc_orbital"]["l2"]["W"]))
    ws = _np(params["enc_signature"]["l1"]["W"])  # [256, 128]
    put("e1s_a", ws[0:128])
    put("e1s_b", ws[128:256])
    put("e2s", _np(params["enc_signature"]["l2"]["W"]))
    for f in range(3):
        blk = np.zeros((128, 128), np.float32)
        fw = _np(params["flows"][f]["W"])
        blk[0:64, 0:64] = fw
        blk[64:128, 64:128] = fw
        put(f"fw{f}", blk)
    ones2 = np.zeros((128, 2), np.float32)
    ones2[0:64, 0] = 1.0
    ones2[64:128, 1] = 1.0
    put("ones2", ones2)
    key = {"physical": "p", "orbital": "o", "signature": "s", "temporal": "t"}
    for m in MODS:
        k = key[m]
        put(f"d1{k}", _np(params["dec_" + m]["l1"]["W"]))
        put(f"d2{k}", _np(params["dec_" + m]["l2"]["W"]))
        put(f"s1{k}", _np(params["score_" + m]["l1"]["W"]))
        put(f"s2{k}", _np(params["score_" + m]["l2"]["W"]))

    # ---- param blob ----
    b1 = _np(lstm[0]["bih"]) + _np(lstm[0]["bhh"])  # [512]
    b2 = _np(lstm[1]["bih"]) + _np(lstm[1]["bhh"])
    for g in range(4):
        pb[:, PB_L1G + g] = b1[g * 128:(g + 1) * 128]
        pb[:, PB_L2G + g] = b2[g * 128:(g + 1) * 128]
    pb[:, PB_E1["physical"]] = _np(params["enc_physical"]["l1"]["b"])
    pb[:, PB_E1["orbital"]] = _np(params["enc_orbital"]["l1"]["b"])
    pb[:, PB_E1["signature"]] = _np(params["enc_signature"]["l1"]["b"])
    pb[0:64, PB_ZA] = _np(params["enc_physical"]["l2"]["b"])
    pb[64:128, PB_ZA] = _np(params["enc_orbital"]["l2"]["b"])
    pb[0:64, PB_ZB] = _np(params["enc_signature"]["l2"]["b"])
    pb[64:128, PB_ZB] = _np(params["temporal_proj"]["b"])
    for f in range(3):
        bf = _np(params["flows"][f]["b"])
        pb[0:64, PB_FLOW + f] = bf
        pb[64:128, PB_FLOW + f] = bf
    for mi, m in enumerate(MODS):
        pb[:, PB_D1 + mi] = _np(params["dec_" + m]["l1"]["b"])
        pb[:, PB_S1 + mi] = _np(params["score_" + m]["l1"]["b"])
        pb[0, PB_S2 + mi] = _np(params["score_" + m]["l2"]["b"])[0]
    pb[0:32, PB_D2["physical"]] = _np(params["dec_physical"]["l2"]["b"])
    pb[0:6, PB_D2["orbital"]] = _np(params["dec_orbital"]["l2"]["b"])
    bs = _np(params["dec_signature"]["l2"]["b"])
    pb[:, PB_D2["signature"]] = bs[0:128]
    pb[:, PB_D2["signature"] + 1] = bs[128:256]
    pb[0:16, PB_D2["temporal"]] = _np(params["dec_temporal"]["l2"]["b"])

    sum_lw = 0.0
    for f in range(3):
        sum_lw += np.linalg.slogdet(_np(params["flows"][f]["W"]))[1]
    pb[0:2, PB_LDW] = sum_lw

    return wb.astype(ml_dtypes.bfloat16), pb


# ==================================================================
# Device program
# ==================================================================
def build_program(t_steps=T):
    nc = bacc.Bacc("TRN2", target_bir_lowering=False, debug=False,
                   num_devices=NCORES)

    XT = nc.declare_dram_parameter("xt", [16, 128, S], BF16, isOutput=False)
    XSIG = nc.declare_dram_parameter("xsig", [2, 128, S], BF16, isOutput=False)
    XPH = nc.declare_dram_parameter("xph", [32, S], BF16, isOutput=False)
    XOR = nc.declare_dram_parameter("xor", [6, S], BF16, isOutput=False)
    WB = nc.declare_dram_parameter("wb", [128, WCOL], BF16, isOutput=False)
    PBIN = nc.declare_dram_parameter("pbin", [128, PCOL], F32, isOutput=False)
    OUT = nc.declare_dram_parameter("out", [NROWS, S], F32, isOutput=True)

    H = S // 2          # 1024: psum tile width / half-block size

    from contextlib import ExitStack
    with TileContext(nc) as tc, ExitStack() as ctx:
        wpool = ctx.enter_context(tc.tile_pool(name="wpool", bufs=1))
        xpool = ctx.enter_context(tc.tile_pool(name="xpool", bufs=6))
        spool = ctx.enter_context(tc.tile_pool(name="spool", bufs=2))
        gpool = ctx.enter_context(tc.tile_pool(name="gpool", bufs=2))
        bpool = ctx.enter_context(tc.tile_pool(name="bpool", bufs=2))
        fpool = ctx.enter_context(tc.tile_pool(name="fpool", bufs=2))
        pspool = ctx.enter_context(tc.tile_pool(name="pspool", bufs=4, space="PSUM"))

        wb = wpool.tile([128, WCOL], BF16, tag="wb")
        pb = wpool.tile([128, PCOL], F32, tag="pb")
        nc.sync.dma_start(out=wb[:], in_=WB[:])
        nc.sync.dma_start(out=pb[:], in_=PBIN[:])

        def wap(name):
            r0, nr, c0, ncol = WSPEC[name]
            return wb[r0:r0 + nr, c0:c0 + ncol]

        def bias(col):
            return pb[:, col:col + 1]

        def mm_half(passes, consumer, prow0=0, prows=128):
            """For each sample half: allocate a [128, H] psum tile, run the
            accumulation passes (each pass = (lhsT, rhs, tile_position)) over
            two 512-wide chunks, then hand (half_index, psum_tile) to consumer.
            Pass rhs is a full [K, S] AP; chunks are taken from it."""
            for hf in range(2):
                ps = pspool.tile([128, H], F32, tag="ps")
                pview = ps[prow0:prow0 + prows, :]
                np_ = len(passes)
                for n in range(2):
                    csl = ts(hf * 2 + n, CH)
                    osl = ts(n, CH)
                    for pi, (lhsT, rhs, tp) in enumerate(passes):
                        nc.tensor.matmul(pview[:, osl], lhsT, rhs[:, csl],
                                         start=(pi == 0), stop=(pi == np_ - 1),
                                         tile_position=tp)
                consumer(hf, ps)

        # ---------------- pair block: flows, ld, dec, score ----------------
        def emit_pair(pair, zf, mods):
            """zf: SBUF f32 [128, S] tile holding the two encodings (biased).
            mods: [(mod_name, global_mod_index), ...]."""
            zb = bpool.tile([128, S], BF16, tag="zb")
            nc.vector.tensor_copy(out=zb[:], in_=zf[:])

            z2s = []
            for f in range(3):
                zb2 = bpool.tile([128, S], BF16, tag="zb")
                z2 = bpool.tile([128, S], BF16, tag=f"z2{f}", bufs=1)

                def flow_cons(hf, ps, zb2=zb2, z2=z2, f=f):
                    hsl = ts(hf, H)
                    nc.scalar.activation(out=zb2[:, hsl], in_=ps[:],
                                         func=AF.Tanh, bias=bias(PB_FLOW + f))
                    nc.vector.tensor_mul(z2[:, hsl], zb2[:, hsl], zb2[:, hsl])

                mm_half([(wap(f"fw{f}"), zb, None)], flow_cons)
                z2s.append(z2)
                zb = zb2

            trf = fpool.tile([128, S], F32, tag="zf")
            nc.vector.tensor_copy(out=trf[:], in_=zb[:])
            nc.sync.dma_start(out=OUT[ROW_TRF + 128 * pair:ROW_TRF + 128 * (pair + 1), :],
                              in_=trf[:])

            # log-det: sum_k log1p(-z^2) over flows, + sum log|det W| (pb col)
            for f in range(3):
                nc.scalar.activation(out=z2s[f][:], in_=z2s[f][:], func=AF.Ln,
                                     bias=1.0, scale=-1.0)
            nc.vector.tensor_add(z2s[0][:], z2s[0][:], z2s[1][:])
            nc.vector.tensor_add(z2s[0][:], z2s[0][:], z2s[2][:])
            ldf = fpool.tile([2, S], F32, tag="ost")

            def ld_cons(hf, ps):
                nc.vector.tensor_scalar_add(ldf[:, ts(hf, H)], ps[0:2, :],
                                            pb[0:2, PB_LDW:PB_LDW + 1])

            mm_half([(wap("ones2"), z2s[0], None)], ld_cons, prows=2)
            nc.sync.dma_start(out=OUT[ROW_LD + 2 * pair:ROW_LD + 2 * pair + 2, :],
                              in_=ldf[:])

            key = {"physical": "p", "orbital": "o", "signature": "s",
                   "temporal": "t"}
            for mi, (m, gmi) in enumerate(mods):
                k = key[m]
                zin = zb[64 * mi:64 * (mi + 1), :]
                # dec l1
                hd = bpool.tile([128, S], BF16, tag="mh")

                def d1_cons(hf, ps, hd=hd, gmi=gmi):
                    nc.scalar.activation(out=hd[:, ts(hf, H)], in_=ps[:],
                                         func=AF.Relu, bias=bias(PB_D1 + gmi))

                mm_half([(wap(f"d1{k}"), zin, None)], d1_cons)
                # dec l2
                D = DIMS[m]
                r0 = REC_OFF[m]
                nchunk = (D + 127) // 128
                for ci in range(nchunk):
                    nr = min(128, D - 128 * ci)
                    _, _, c0, _ = WSPEC[f"d2{k}"]
                    lhsT = wb[0:128, c0 + 128 * ci:c0 + 128 * ci + nr]
                    ost = fpool.tile([128, S], F32, tag="ost")

                    def d2_cons(hf, ps, ost=ost, nr=nr, m=m, ci=ci):
                        nc.vector.tensor_scalar_add(
                            ost[0:nr, ts(hf, H)], ps[0:nr, :],
                            pb[0:nr, PB_D2[m] + ci:PB_D2[m] + ci + 1])

                    mm_half([(lhsT, hd, None)], d2_cons, prows=nr)
                    nc.sync.dma_start(out=OUT[r0 + 128 * ci:r0 + 128 * ci + nr, :],
                                      in_=ost[0:nr, :])
                # score l1
                hs = bpool.tile([128, S], BF16, tag="mh")

                def s1_cons(hf, ps, hs=hs, gmi=gmi):
                    nc.scalar.activation(out=hs[:, ts(hf, H)], in_=ps[:],
                                         func=AF.Relu, bias=bias(PB_S1 + gmi))

                mm_half([(wap(f"s1{k}"), zin, None)], s1_cons)
                # score l2 + sigmoid
                scf = fpool.tile([1, S], F32, tag="ost")

                def s2_cons(hf, ps, scf=scf, gmi=gmi):
                    nc.scalar.activation(out=scf[:, ts(hf, H)], in_=ps[0:1, :],
                                         func=AF.Sigmoid,
                                         bias=pb[0:1, PB_S2 + gmi:PB_S2 + gmi + 1])

                mm_half([(wap(f"s2{k}"), hs, None)], s2_cons, prows=1)
                nc.sync.dma_start(out=OUT[ROW_SC + gmi:ROW_SC + gmi + 1, :],
                                  in_=scf[:])

        # ---------------- pair A prelude: phys + orb encoders --------------
        xph = bpool.tile([32, S], BF16, tag="xin")
        nc.sync.dma_start(out=xph[:], in_=XPH[:])
        xorb = bpool.tile([6, S], BF16, tag="xin")
        nc.sync.dma_start(out=xorb[:], in_=XOR[:])

        hp = bpool.tile([128, S], BF16, tag="mh")
        mm_half([(wap("e1p"), xph, None)],
                lambda hf, ps: nc.scalar.activation(
                    out=hp[:, ts(hf, H)], in_=ps[:], func=AF.Relu,
                    bias=bias(PB_E1["physical"])))
        ho = bpool.tile([128, S], BF16, tag="mh")
        mm_half([(wap("e1o"), xorb, None)],
                lambda hf, ps: nc.scalar.activation(
                    out=ho[:, ts(hf, H)], in_=ps[:], func=AF.Relu,
                    bias=bias(PB_E1["orbital"])))

        zfA = fpool.tile([128, S], F32, tag="zf")
        for hf in range(2):
            ps = pspool.tile([128, H], F32, tag="ps")
            for n in range(2):
                csl = ts(hf * 2 + n, CH)
                osl = ts(n, CH)
                nc.tensor.matmul(ps[0:64, osl], wap("e2p"), hp[:, csl],
                                 start=True, stop=True)
                nc.tensor.matmul(ps[64:128, osl], wap("e2o"), ho[:, csl],
                                 start=True, stop=True, tile_position=(0, 64))
            nc.vector.tensor_scalar_add(zfA[:, ts(hf, H)], ps[:], bias(PB_ZA))
        nc.sync.dma_start(out=OUT[ROW_ENC:ROW_ENC + 128, :], in_=zfA[:])

        emit_pair(0, zfA, [("physical", 0), ("orbital", 1)])

        # ---------------- signature encoder (independent of LSTM) ----------
        xs0 = bpool.tile([128, S], BF16, tag="xin")
        xs1 = bpool.tile([128, S], BF16, tag="xin")
        nc.sync.dma_start(out=xs0[:], in_=XSIG[0])
        nc.sync.dma_start(out=xs1[:], in_=XSIG[1])
        hsg = bpool.tile([128, S], BF16, tag="mh")
        mm_half([(wap("e1s_a"), xs0, None), (wap("e1s_b"), xs1, None)],
                lambda hf, ps: nc.scalar.activation(
                    out=hsg[:, ts(hf, H)], in_=ps[:], func=AF.Relu,
                    bias=bias(PB_E1["signature"])))
        zfB = fpool.tile([128, S], F32, tag="zfB", bufs=1)

        def zsig_cons(hf, ps):
            nc.vector.tensor_scalar_add(zfB[0:64, ts(hf, H)], ps[0:64, :],
                                        pb[0:64, PB_ZB:PB_ZB + 1])

        mm_half([(wap("e2s"), hsg, None)], zsig_cons, prows=64)

        # ---------------- LSTM ----------------
        h1 = spool.tile([128, S], BF16, tag="h1")
        c1 = spool.tile([128, S], BF16, tag="c1")
        h2 = spool.tile([128, S], BF16, tag="h2")
        c2 = spool.tile([128, S], BF16, tag="c2")
        for t0 in (h1, c1, h2, c2):
            nc.vector.memset(t0[:], 0.0)

        def lstm_layer(x_ap, wih_name, wih_r0, whh_name, bias0, prev_h, prev_c,
                       h_tag, c_tag, hh_first):
            # hh_first: for layer 2, h2(t-1) is ready long before h1(t), so
            # the Whh pass goes first to keep the PE busy during layer 1's
            # cell-update chain.
            gates = {}
            _, _, cih, _ = WSPEC[wih_name]
            _, _, chh, _ = WSPEC[whh_name]
            kin = x_ap.shape[0]
            tp_ih = (96, 0) if wih_r0 == 96 else None
            for gi, nm in enumerate("ifgo"):
                lih = wb[wih_r0:wih_r0 + kin, cih + 128 * gi:cih + 128 * (gi + 1)]
                lhh = wb[0:128, chh + 128 * gi:chh + 128 * (gi + 1)]
                g_sb = gpool.tile([128, S], BF16, tag="g" + nm)
                if hh_first:
                    passes = [(lhh, prev_h, None), (lih, x_ap, tp_ih)]
                else:
                    passes = [(lih, x_ap, tp_ih), (lhh, prev_h, None)]

                def g_cons(hf, ps, g_sb=g_sb, nm=nm, gi=gi):
                    nc.scalar.activation(out=g_sb[:, ts(hf, H)], in_=ps[:],
                                         func=AF.Tanh if nm == "g" else AF.Sigmoid,
                                         bias=bias(bias0 + gi))

                mm_half(passes, g_cons)
                gates[nm] = g_sb
            # cell update in two sample halves so ACT/DVE pipeline
            c_new = spool.tile([128, S], BF16, tag=c_tag)
            tc_sb = gpool.tile([128, S], BF16, tag="tc")
            h_new = spool.tile([128, S], BF16, tag=h_tag)
            for hf in range(2):
                sl = ts(hf, H)
                nc.vector.tensor_mul(gates["f"][:, sl], gates["f"][:, sl],
                                     prev_c[:, sl])
                nc.vector.tensor_mul(gates["i"][:, sl], gates["i"][:, sl],
                                     gates["g"][:, sl])
                nc.vector.tensor_add(c_new[:, sl], gates["f"][:, sl],
                                     gates["i"][:, sl])
                nc.scalar.activation(out=tc_sb[:, sl], in_=c_new[:, sl],
                                     func=AF.Tanh)
                nc.vector.tensor_mul(h_new[:, sl], gates["o"][:, sl],
                                     tc_sb[:, sl])
            return h_new, c_new

        xt_tile = None
        for t in range(t_steps):
            j, r = divmod(t, 4)
            if r == 0:
                xt_tile = xpool.tile([128, S], BF16, tag="xt")
                nc.sync.dma_start(out=xt_tile[:], in_=XT[j])
            x_ap = xt_tile[32 * r:32 * r + 16, :]
            h1, c1 = lstm_layer(x_ap, "wih1", 32 * r, "whh1", PB_L1G,
                                h1, c1, "h1", "c1", hh_first=False)
            h2, c2 = lstm_layer(h1, "wih2", 0, "whh2", PB_L2G,
                                h2, c2, "h2", "c2", hh_first=True)

        # ---------------- pair B: temporal projection + pair block ---------
        def ztp_cons(hf, ps):
            nc.vector.tensor_scalar_add(zfB[64:128, ts(hf, H)], ps[64:128, :],
                                        pb[64:128, PB_ZB:PB_ZB + 1])

        mm_half([(wap("wtp"), h2, (0, 64))], ztp_cons, prow0=64, prows=64)
        nc.sync.dma_start(out=OUT[ROW_ENC + 128:ROW_ENC + 256, :], in_=zfB[:])

        emit_pair(1, zfB, [("signature", 2), ("temporal", 3)])

    nc.compile()
    return nc


# ==================================================================
# Host wrapper
# ==================================================================
_CACHE = {}


def _prep_core_inputs(x_physical, x_orbital, x_signature, x_temporal, wb, pbin):
    bf16 = ml_dtypes.bfloat16
    in_maps = []
    for c in range(NCORES):
        sl = slice(c * S, (c + 1) * S)
        xt = np.transpose(x_temporal[sl], (1, 2, 0))          # [64, 16, S]
        xt = np.ascontiguousarray(xt).reshape(16, 4, 16, S)
        pad = np.zeros((16, 4, 32, S), np.float32)
        pad[:, :, :16, :] = xt
        XTc = pad.reshape(16, 128, S).astype(bf16)
        XSIGc = np.ascontiguousarray(x_signature[sl].T).reshape(2, 128, S).astype(bf16)
        XPHc = np.ascontiguousarray(x_physical[sl].T).astype(bf16)
        XORc = np.ascontiguousarray(x_orbital[sl].T).astype(bf16)
        in_maps.append({
            "xt": XTc, "xsig": XSIGc, "xph": XPHc, "xor": XORc,
            "wb": wb, "pbin": pbin,
        })
    return in_maps


LAST_RESULT = None


def kernel(x_physical, x_orbital, x_signature, x_temporal, params,
           _trace=False, _trace_kwargs=None):
    global LAST_RESULT
    x_physical = _np(x_physical)
    x_orbital = _np(x_orbital)
    x_signature = _np(x_signature)
    x_temporal = _np(x_temporal)

    wb, pbin = build_blobs(params)

    key = "prog"
    if key not in _CACHE:
        _CACHE[key] = build_program()
    nc = _CACHE[key]

    in_maps = _prep_core_inputs(x_physical, x_orbital, x_signature, x_temporal,
                                wb, pbin)
    res = run_bass_kernel_spmd(nc, in_maps, list(range(NCORES)),
                               trace=_trace, **(_trace_kwargs or {}))
    LAST_RESULT = res

    O = np.concatenate([res.results[c]["out"] for c in range(NCORES)], axis=1)

    def rows(r0, n):
        return np.ascontiguousarray(O[r0:r0 + n].T)

    enc = {}
    trf = {}
    rec = {}
    ld = {}
    sc = {}
    for mi, m in enumerate(MODS):
        enc[m] = rows(ROW_ENC + 64 * mi, 64)
        trf[m] = rows(ROW_TRF + 64 * mi, 64)
        rec[m] = rows(REC_OFF[m], DIMS[m])
        ld[m] = np.ascontiguousarray(O[ROW_LD + mi])
        sc[m] = rows(ROW_SC + mi, 1)
    return {
        "encodings": enc,
        "transformed": trf,
        "reconstructions": rec,
        "log_det": ld,
        "anomaly_scores": sc,
    }


# revision 9
# speedup vs baseline: 1.0793x; 1.0793x over previous
"""Trainium2 Bass kernel for nn_AnomalyDetector (multi-modal encoder + 2-layer
LSTM + normalizing flows + decoders + anomaly scores).

Data-parallel over 8 NeuronCores: batch 16384 -> 2048 per core. All on-chip
activations use a transposed layout [feature_on_partition, sample_on_free] so
the per-step LSTM matmuls need no transposes. Compute in bf16 with f32 PSUM
accumulation; outputs staged to one [830, 2048] f32 DRAM tensor per core.
"""

import numpy as np
import ml_dtypes

import concourse.bass as bass
import concourse.mybir as mybir
from concourse import bacc
from concourse.tile import TileContext
from concourse.bass import ts
from concourse.bass_utils import run_bass_kernel_spmd

BF16 = mybir.dt.bfloat16
F32 = mybir.dt.float32
AF = mybir.ActivationFunctionType

B = 16384
NCORES = 8
S = B // NCORES          # samples per core (2048)
T = 64
HID = 128
LAT = 64
DIMS = {"physical": 32, "orbital": 6, "signature": 256, "temporal": 16}
MODS = ["physical", "orbital", "signature", "temporal"]
NCH = 4                  # moving-sample chunks of 512
CH = S // NCH            # 512

# ------------------------------------------------------------------
# Output row map for the per-core OUT tensor [830, S] (feature, sample)
# ------------------------------------------------------------------
ROW_ENC = 0        # 4 x 64  (phys, orb, sig, temp)
ROW_TRF = 256      # 4 x 64
ROW_REC = 512      # 32, 6, 256, 16
REC_OFF = {"physical": 512, "orbital": 544, "signature": 550, "temporal": 806}
ROW_LD = 822       # 4 rows (phys, orb, sig, temp)
ROW_SC = 826       # 4 rows
NROWS = 830

# ------------------------------------------------------------------
# Weight blob (bf16) / param blob (f32) layouts: name -> (r0, nr, c0, nc)
# ------------------------------------------------------------------
def _build_wspec():
    spec = {}
    col = 0

    def add(name, r0, nr, nc):
        nonlocal col
        spec[name] = (r0, nr, col, nc)
        col += nc

    add("wih1", 0, 128, 512)      # 4x replicated along 32-row groups
    add("whh1", 0, 128, 512)
    add("wih2", 0, 128, 512)
    add("whh2", 0, 128, 512)
    add("wtp", 0, 128, 64)
    add("e1p", 0, 32, 128)
    add("e2p", 0, 128, 64)
    add("e1o", 0, 6, 128)
    add("e2o", 0, 128, 64)
    add("e1s_a", 0, 128, 128)
    add("e1s_b", 0, 128, 128)
    add("e2s", 0, 128, 64)
    add("fw0", 0, 128, 128)
    add("fw1", 0, 128, 128)
    add("fw2", 0, 128, 128)
    add("ones2", 0, 128, 2)
    add("d1p", 0, 64, 128)
    add("d1o", 64, 64, 128)
    add("d1s", 0, 64, 128)
    add("d1t", 64, 64, 128)
    add("d2p", 0, 128, 32)
    add("d2o", 0, 128, 6)
    add("d2s", 0, 128, 256)
    add("d2t", 0, 128, 16)
    add("s1p", 0, 64, 128)
    add("s1o", 64, 64, 128)
    add("s1s", 0, 64, 128)
    add("s1t", 64, 64, 128)
    add("s2p", 0, 128, 1)
    add("s2o", 0, 128, 1)
    add("s2s", 0, 128, 1)
    add("s2t", 0, 128, 1)
    return spec, col


WSPEC, WCOL = _build_wspec()

# param blob columns (f32)
PB_L1G = 0      # cols 0-3: layer1 gate biases (i,f,g,o)
PB_L2G = 4      # cols 4-7
PB_E1 = {"physical": 8, "orbital": 9, "signature": 10}
PB_ZA = 11      # [b2_phys ; b2_orb]
PB_ZB = 12      # [b2_sig ; b_tp]
PB_FLOW = 13    # 13,14,15
PB_D1 = 16      # 16-19 dec l1 biases (p,o,s,t)
PB_S1 = 20      # 20-23 score l1 biases
PB_D2 = {"physical": 24, "orbital": 25, "signature": 26, "temporal": 28}  # sig uses 26,27
PB_S2 = 29      # 29-32, row 0 only
PB_LDW = 33     # rows 0-1: sum of flow log|det W|
PCOL = 34


def _np(x):
    return np.asarray(x, dtype=np.float32)


def build_blobs(params):
    """Pack all model parameters into WBLOB (bf16) and PBLOB (f32)."""
    wb = np.zeros((128, WCOL), np.float32)
    pb = np.zeros((128, PCOL), np.float32)

    def put(name, arr):
        r0, nr, c0, ncol = WSPEC[name]
        a = _np(arr)
        assert a.shape == (nr, ncol), (name, a.shape, (nr, ncol))
        wb[r0:r0 + nr, c0:c0 + ncol] = a

    lstm = params["lstm"]
    # layer 1 Wih replicated at row groups 0..3 (16 rows each, 32-aligned)
    wih1 = np.zeros((128, 512), np.float32)
    w = _np(lstm[0]["Wih"])  # [16, 512]
    for r in range(4):
        wih1[32 * r:32 * r + 16, :] = w
    put("wih1", wih1)
    put("whh1", _np(lstm[0]["Whh"]))
    put("wih2", _np(lstm[1]["Wih"]))
    put("whh2", _np(lstm[1]["Whh"]))
    put("wtp", _np(params["temporal_proj"]["W"]))
    put("e1p", _np(params["enc_physical"]["l1"]["W"]))
    put("e2p", _np(params["enc_physical"]["l2"]["W"]))
    put("e1o", _np(params["enc_orbital"]["l1"]["W"]))
    put("e2o", _np(params["enc_orbital"]["l2"]["W"]))
    ws = _np(params["enc_signature"]["l1"]["W"])  # [256, 128]
    put("e1s_a", ws[0:128])
    put("e1s_b", ws[128:256])
    put("e2s", _np(params["enc_signature"]["l2"]["W"]))
    for f in range(3):
        blk = np.zeros((128, 128), np.float32)
        fw = _np(params["flows"][f]["W"])
        blk[0:64, 0:64] = fw
        blk[64:128, 64:128] = fw
        put(f"fw{f}", blk)
    ones2 = np.zeros((128, 2), np.float32)
    ones2[0:64, 0] = 1.0
    ones2[64:128, 1] = 1.0
    put("ones2", ones2)
    key = {"physical": "p", "orbital": "o", "signature": "s", "temporal": "t"}
    for m in MODS:
        k = key[m]
        put(f"d1{k}", _np(params["dec_" + m]["l1"]["W"]))
        put(f"d2{k}", _np(params["dec_" + m]["l2"]["W"]))
        put(f"s1{k}", _np(params["score_" + m]["l1"]["W"]))
        put(f"s2{k}", _np(params["score_" + m]["l2"]["W"]))

    # ---- param blob ----
    b1 = _np(lstm[0]["bih"]) + _np(lstm[0]["bhh"])  # [512]
    b2 = _np(lstm[1]["bih"]) + _np(lstm[1]["bhh"])
    for g in range(4):
        pb[:, PB_L1G + g] = b1[g * 128:(g + 1) * 128]
        pb[:, PB_L2G + g] = b2[g * 128:(g + 1) * 128]
    pb[:, PB_E1["physical"]] = _np(params["enc_physical"]["l1"]["b"])
    pb[:, PB_E1["orbital"]] = _np(params["enc_orbital"]["l1"]["b"])
    pb[:, PB_E1["signature"]] = _np(params["enc_signature"]["l1"]["b"])
    pb[0:64, PB_ZA] = _np(params["enc_physical"]["l2"]["b"])
    pb[64:128, PB_ZA] = _np(params["enc_orbital"]["l2"]["b"])
    pb[0:64, PB_ZB] = _np(params["enc_signature"]["l2"]["b"])
    pb[64:128, PB_ZB] = _np(params["temporal_proj"]["b"])
    for f in range(3):
        bf = _np(params["flows"][f]["b"])
        pb[0:64, PB_FLOW + f] = bf
        pb[64:128, PB_FLOW + f] = bf
    for mi, m in enumerate(MODS):
        pb[:, PB_D1 + mi] = _np(params["dec_" + m]["l1"]["b"])
        pb[:, PB_S1 + mi] = _np(params["score_" + m]["l1"]["b"])
        pb[0, PB_S2 + mi] = _np(params["score_" + m]["l2"]["b"])[0]
    pb[0:32, PB_D2["physical"]] = _np(params["dec_physical"]["l2"]["b"])
    pb[0:6, PB_D2["orbital"]] = _np(params["dec_orbital"]["l2"]["b"])
    bs = _np(params["dec_signature"]["l2"]["b"])
    pb[:, PB_D2["signature"]] = bs[0:128]
    pb[:, PB_D2["signature"] + 1] = bs[128:256]
    pb[0:16, PB_D2["temporal"]] = _np(params["dec_temporal"]["l2"]["b"])

    sum_lw = 0.0
    for f in range(3):
        sum_lw += np.linalg.slogdet(_np(params["flows"][f]["W"]))[1]
    pb[0:2, PB_LDW] = sum_lw

    return wb.astype(ml_dtypes.bfloat16), pb


# ==================================================================
# Device program
# ==================================================================
def build_program(t_steps=T):
    nc = bacc.Bacc("TRN2", target_bir_lowering=False, debug=False,
                   num_devices=NCORES)

    XT = nc.declare_dram_parameter("xt", [16, 128, S], BF16, isOutput=False)
    XSIG = nc.declare_dram_parameter("xsig", [2, 128, S], BF16, isOutput=False)
    XPH = nc.declare_dram_parameter("xph", [32, S], BF16, isOutput=False)
    XOR = nc.declare_dram_parameter("xor", [6, S], BF16, isOutput=False)
    WB = nc.declare_dram_parameter("wb", [128, WCOL], BF16, isOutput=False)
    PBIN = nc.declare_dram_parameter("pbin", [128, PCOL], F32, isOutput=False)
    OUT = nc.declare_dram_parameter("out", [NROWS, S], F32, isOutput=True)

    H = S // 2          # 1024: bf16 moving-chunk width / half-block size

    from contextlib import ExitStack
    with TileContext(nc) as tc, ExitStack() as ctx:
        wpool = ctx.enter_context(tc.tile_pool(name="wpool", bufs=1))
        xpool = ctx.enter_context(tc.tile_pool(name="xpool", bufs=6))
        spool = ctx.enter_context(tc.tile_pool(name="spool", bufs=2))
        gpool = ctx.enter_context(tc.tile_pool(name="gpool", bufs=2))
        bpool = ctx.enter_context(tc.tile_pool(name="bpool", bufs=2))
        fpool = ctx.enter_context(tc.tile_pool(name="fpool", bufs=2))
        pspool = ctx.enter_context(tc.tile_pool(name="pspool", bufs=2, space="PSUM"))

        wb = wpool.tile([128, WCOL], BF16, tag="wb")
        pb = wpool.tile([128, PCOL], F32, tag="pb")
        nc.sync.dma_start(out=wb[:], in_=WB[:])
        nc.sync.dma_start(out=pb[:], in_=PBIN[:])

        def wap(name):
            r0, nr, c0, ncol = WSPEC[name]
            return wb[r0:r0 + nr, c0:c0 + ncol]

        def bias(col):
            return pb[:, col:col + 1]

        def mm_chain(ps_ap, passes):
            """Accumulate passes (lhsT, rhs, tile_position) into ps_ap over
            two bf16 moving chunks of 1024."""
            np_ = len(passes)
            for n in range(NCH):
                nsl = ts(n, CH)
                for pi, (lhsT, rhs, tp) in enumerate(passes):
                    nc.tensor.matmul(ps_ap[:, nsl], lhsT, rhs[:, nsl],
                                     start=(pi == 0), stop=(pi == np_ - 1),
                                     tile_position=tp)

        # ---------------- pair block: flows, ld, dec, score ----------------
        def emit_pair(pair, zf, mods):
            """zf: SBUF f32 [128, S] tile holding the two encodings (biased)."""
            zb = bpool.tile([128, S], BF16, tag="zb")
            nc.vector.tensor_copy(out=zb[:], in_=zf[:])

            z2s = []
            for f in range(3):
                ps = pspool.tile([128, S], F32, tag="ps")
                mm_chain(ps, [(wap(f"fw{f}"), zb, None)])
                zb2 = bpool.tile([128, S], BF16, tag="zb")
                nc.scalar.activation(out=zb2[:], in_=ps[:], func=AF.Tanh,
                                     bias=bias(PB_FLOW + f))
                z2 = bpool.tile([128, S], BF16, tag=f"z2{f}", bufs=1)
                nc.vector.tensor_mul(z2[:], zb2[:], zb2[:])
                z2s.append(z2)
                zb = zb2

            trf = fpool.tile([128, S], F32, tag="zf")
            nc.vector.tensor_copy(out=trf[:], in_=zb[:])
            nc.sync.dma_start(out=OUT[ROW_TRF + 128 * pair:ROW_TRF + 128 * (pair + 1), :],
                              in_=trf[:])

            for f in range(3):
                nc.scalar.activation(out=z2s[f][:], in_=z2s[f][:], func=AF.Ln,
                                     bias=1.0, scale=-1.0)
            nc.vector.tensor_add(z2s[0][:], z2s[0][:], z2s[1][:])
            nc.vector.tensor_add(z2s[0][:], z2s[0][:], z2s[2][:])
            psl = pspool.tile([128, S], F32, tag="ps")
            mm_chain(psl[0:2, :], [(wap("ones2"), z2s[0], None)])
            ldf = fpool.tile([2, S], F32, tag="ost")
            nc.vector.tensor_scalar_add(ldf[:], psl[0:2, :],
                                        pb[0:2, PB_LDW:PB_LDW + 1])
            nc.sync.dma_start(out=OUT[ROW_LD + 2 * pair:ROW_LD + 2 * pair + 2, :],
                              in_=ldf[:])

            key = {"physical": "p", "orbital": "o", "signature": "s",
                   "temporal": "t"}
            for mi, (m, gmi) in enumerate(mods):
                k = key[m]
                zin = zb[64 * mi:64 * (mi + 1), :]
                ps = pspool.tile([128, S], F32, tag="ps")
                mm_chain(ps, [(wap(f"d1{k}"), zin, None)])
                hd = bpool.tile([128, S], BF16, tag="mh")
                nc.scalar.activation(out=hd[:], in_=ps[:], func=AF.Relu,
                                     bias=bias(PB_D1 + gmi))
                D = DIMS[m]
                r0 = REC_OFF[m]
                nchunk = (D + 127) // 128
                for ci in range(nchunk):
                    nr = min(128, D - 128 * ci)
                    _, _, c0, _ = WSPEC[f"d2{k}"]
                    lhsT = wb[0:128, c0 + 128 * ci:c0 + 128 * ci + nr]
                    ps2 = pspool.tile([128, S], F32, tag="ps")
                    mm_chain(ps2[0:nr, :], [(lhsT, hd, None)])
                    ost = fpool.tile([128, S], F32, tag="ost")
                    nc.vector.tensor_scalar_add(
                        ost[0:nr, :], ps2[0:nr, :],
                        pb[0:nr, PB_D2[m] + ci:PB_D2[m] + ci + 1])
                    nc.sync.dma_start(out=OUT[r0 + 128 * ci:r0 + 128 * ci + nr, :],
                                      in_=ost[0:nr, :])
                ps3 = pspool.tile([128, S], F32, tag="ps")
                mm_chain(ps3, [(wap(f"s1{k}"), zin, None)])
                hs = bpool.tile([128, S], BF16, tag="mh")
                nc.scalar.activation(out=hs[:], in_=ps3[:], func=AF.Relu,
                                     bias=bias(PB_S1 + gmi))
                ps4 = pspool.tile([128, S], F32, tag="ps")
                mm_chain(ps4[0:1, :], [(wap(f"s2{k}"), hs, None)])
                scf = fpool.tile([1, S], F32, tag="ost")
                nc.scalar.activation(out=scf[:], in_=ps4[0:1, :], func=AF.Sigmoid,
                                     bias=pb[0:1, PB_S2 + gmi:PB_S2 + gmi + 1])
                nc.sync.dma_start(out=OUT[ROW_SC + gmi:ROW_SC + gmi + 1, :],
                                  in_=scf[:])

        # ---------------- pair A prelude: phys + orb encoders --------------
        xph = bpool.tile([32, S], BF16, tag="xin")
        nc.sync.dma_start(out=xph[:], in_=XPH[:])
        xorb = bpool.tile([6, S], BF16, tag="xin")
        nc.sync.dma_start(out=xorb[:], in_=XOR[:])

        psp = pspool.tile([128, S], F32, tag="ps")
        mm_chain(psp, [(wap("e1p"), xph, None)])
        hp = bpool.tile([128, S], BF16, tag="mh")
        nc.scalar.activation(out=hp[:], in_=psp[:], func=AF.Relu,
                             bias=bias(PB_E1["physical"]))
        pso = pspool.tile([128, S], F32, tag="ps")
        mm_chain(pso, [(wap("e1o"), xorb, None)])
        ho = bpool.tile([128, S], BF16, tag="mh")
        nc.scalar.activation(out=ho[:], in_=pso[:], func=AF.Relu,
                             bias=bias(PB_E1["orbital"]))

        zpsA = pspool.tile([128, S], F32, tag="ps")
        mm_chain(zpsA[0:64, :], [(wap("e2p"), hp, None)])
        mm_chain(zpsA[64:128, :], [(wap("e2o"), ho, (0, 64))])
        zfA = fpool.tile([128, S], F32, tag="zf")
        nc.vector.tensor_scalar_add(zfA[:], zpsA[:], bias(PB_ZA))
        nc.sync.dma_start(out=OUT[ROW_ENC:ROW_ENC + 128, :], in_=zfA[:])

        emit_pair(0, zfA, [("physical", 0), ("orbital", 1)])

        # ---------------- signature encoder (independent of LSTM) ----------
        xs0 = bpool.tile([128, S], BF16, tag="xin")
        xs1 = bpool.tile([128, S], BF16, tag="xin")
        nc.sync.dma_start(out=xs0[:], in_=XSIG[0])
        nc.sync.dma_start(out=xs1[:], in_=XSIG[1])
        pss = pspool.tile([128, S], F32, tag="ps")
        mm_chain(pss, [(wap("e1s_a"), xs0, None), (wap("e1s_b"), xs1, None)])
        hsg = bpool.tile([128, S], BF16, tag="mh")
        nc.scalar.activation(out=hsg[:], in_=pss[:], func=AF.Relu,
                             bias=bias(PB_E1["signature"]))
        zfB = fpool.tile([128, S], F32, tag="zfB", bufs=1)
        pssz = pspool.tile([128, S], F32, tag="ps")
        mm_chain(pssz[0:64, :], [(wap("e2s"), hsg, None)])
        nc.vector.tensor_scalar_add(zfB[0:64, :], pssz[0:64, :],
                                    pb[0:64, PB_ZB:PB_ZB + 1])

        # ---------------- LSTM ----------------
        h1 = spool.tile([128, S], BF16, tag="h1")
        c1 = spool.tile([128, S], BF16, tag="c1")
        h2 = spool.tile([128, S], BF16, tag="h2")
        c2 = spool.tile([128, S], BF16, tag="c2")
        for t0 in (h1, c1, h2, c2):
            nc.vector.memset(t0[:], 0.0)

        def lstm_layer(x_ap, wih_name, wih_r0, whh_name, bias0, prev_h, prev_c,
                       h_tag, c_tag, hh_first):
            # hh_first: for layer 2, h2(t-1) is ready long before h1(t), so
            # the Whh pass goes first to keep the PE busy during layer 1's
            # cell-update chain.
            gates = {}
            _, _, cih, _ = WSPEC[wih_name]
            _, _, chh, _ = WSPEC[whh_name]
            kin = x_ap.shape[0]
            tp_ih = (96, 0) if wih_r0 == 96 else None
            for gi, nm in enumerate("ifgo"):
                lih = wb[wih_r0:wih_r0 + kin, cih + 128 * gi:cih + 128 * (gi + 1)]
                lhh = wb[0:128, chh + 128 * gi:chh + 128 * (gi + 1)]
                if hh_first:
                    passes = [(lhh, prev_h, None), (lih, x_ap, tp_ih)]
                else:
                    passes = [(lih, x_ap, tp_ih), (lhh, prev_h, None)]
                ps = pspool.tile([128, S], F32, tag="ps")
                mm_chain(ps, passes)
                g_sb = gpool.tile([128, S], BF16, tag="g" + nm)
                nc.scalar.activation(out=g_sb[:], in_=ps[:],
                                     func=AF.Tanh if nm == "g" else AF.Sigmoid,
                                     bias=bias(bias0 + gi))
                gates[nm] = g_sb
            # cell update: DVE in two halves (pipelines with ACT tanh), tanh
            # full-width (fewer ACT instructions -- ACT is the bottleneck)
            c_new = spool.tile([128, S], BF16, tag=c_tag)
            tc_sb = gpool.tile([128, S], BF16, tag="tc")
            h_new = spool.tile([128, S], BF16, tag=h_tag)
            for hf in range(2):
                sl = ts(hf, H)
                nc.vector.tensor_mul(gates["f"][:, sl], gates["f"][:, sl],
                                     prev_c[:, sl])
                nc.vector.tensor_mul(gates["i"][:, sl], gates["i"][:, sl],
                                     gates["g"][:, sl])
                nc.vector.tensor_add(c_new[:, sl], gates["f"][:, sl],
                                     gates["i"][:, sl])
            nc.scalar.activation(out=tc_sb[:], in_=c_new[:], func=AF.Tanh)
            for hf in range(2):
                sl = ts(hf, H)
                nc.vector.tensor_mul(h_new[:, sl], gates["o"][:, sl],
                                     tc_sb[:, sl])
            return h_new, c_new

        xt_tile = None
        for t in range(t_steps):
            j, r = divmod(t, 4)
            if r == 0:
                xt_tile = xpool.tile([128, S], BF16, tag="xt")
                nc.sync.dma_start(out=xt_tile[:], in_=XT[j])
            x_ap = xt_tile[32 * r:32 * r + 16, :]
            h1, c1 = lstm_layer(x_ap, "wih1", 32 * r, "whh1", PB_L1G,
                                h1, c1, "h1", "c1", hh_first=False)
            h2, c2 = lstm_layer(h1, "wih2", 0, "whh2", PB_L2G,
                                h2, c2, "h2", "c2", hh_first=True)

        # ---------------- pair B: temporal projection + pair block ---------
        pst = pspool.tile([128, S], F32, tag="ps")
        mm_chain(pst[64:128, :], [(wap("wtp"), h2, (0, 64))])
        nc.vector.tensor_scalar_add(zfB[64:128, :], pst[64:128, :],
                                    pb[64:128, PB_ZB:PB_ZB + 1])
        nc.sync.dma_start(out=OUT[ROW_ENC + 128:ROW_ENC + 256, :], in_=zfB[:])

        emit_pair(1, zfB, [("signature", 2), ("temporal", 3)])

    nc.compile()
    return nc


# ==================================================================
# Host wrapper
# ==================================================================
_CACHE = {}


def _prep_core_inputs(x_physical, x_orbital, x_signature, x_temporal, wb, pbin):
    bf16 = ml_dtypes.bfloat16
    in_maps = []
    for c in range(NCORES):
        sl = slice(c * S, (c + 1) * S)
        xt = np.transpose(x_temporal[sl], (1, 2, 0))          # [64, 16, S]
        xt = np.ascontiguousarray(xt).reshape(16, 4, 16, S)
        pad = np.zeros((16, 4, 32, S), np.float32)
        pad[:, :, :16, :] = xt
        XTc = pad.reshape(16, 128, S).astype(bf16)
        XSIGc = np.ascontiguousarray(x_signature[sl].T).reshape(2, 128, S).astype(bf16)
        XPHc = np.ascontiguousarray(x_physical[sl].T).astype(bf16)
        XORc = np.ascontiguousarray(x_orbital[sl].T).astype(bf16)
        in_maps.append({
            "xt": XTc, "xsig": XSIGc, "xph": XPHc, "xor": XORc,
            "wb": wb, "pbin": pbin,
        })
    return in_maps


LAST_RESULT = None


def kernel(x_physical, x_orbital, x_signature, x_temporal, params,
           _trace=False, _trace_kwargs=None):
    global LAST_RESULT
    x_physical = _np(x_physical)
    x_orbital = _np(x_orbital)
    x_signature = _np(x_signature)
    x_temporal = _np(x_temporal)

    wb, pbin = build_blobs(params)

    key = "prog"
    if key not in _CACHE:
        _CACHE[key] = build_program()
    nc = _CACHE[key]

    in_maps = _prep_core_inputs(x_physical, x_orbital, x_signature, x_temporal,
                                wb, pbin)
    res = run_bass_kernel_spmd(nc, in_maps, list(range(NCORES)),
                               trace=_trace, **(_trace_kwargs or {}))
    LAST_RESULT = res

    O = np.concatenate([res.results[c]["out"] for c in range(NCORES)], axis=1)

    def rows(r0, n):
        return np.ascontiguousarray(O[r0:r0 + n].T)

    enc = {}
    trf = {}
    rec = {}
    ld = {}
    sc = {}
    for mi, m in enumerate(MODS):
        enc[m] = rows(ROW_ENC + 64 * mi, 64)
        trf[m] = rows(ROW_TRF + 64 * mi, 64)
        rec[m] = rows(REC_OFF[m], DIMS[m])
        ld[m] = np.ascontiguousarray(O[ROW_LD + mi])
        sc[m] = rows(ROW_SC + mi, 1)
    return {
        "encodings": enc,
        "transformed": trf,
        "reconstructions": rec,
        "log_det": ld,
        "anomaly_scores": sc,
    }
